# revision 26
# baseline (speedup 1.0000x reference)
"""Trainium2 Bass kernel: conv2d(3x3,VALID) + bias -> min over C_out -> tanh(tanh).

Full-input contract: kernel(**inputs) takes the unsharded inputs
  x:           [32, 16, 256, 256] f32
  conv_weight: [64, 16, 3, 3]     f32
  conv_bias:   [64]               f32
and returns [32, 1, 254, 254] f32.

Strategy (data-parallel over batch, 4 images per core on 8 cores):
conv as matmuls with J=4 position shifts per moving column group.  The key
byte-saving: with J=4 the stationary columns are flat positions t with
t % 4 == 0 (kh offsets are 256 = 4*64, so every kh tap stays on the same
residue), so the SBUF slab only stores every 4th image column:
  slab[kw*16+c, u] = x[c, 4u + kw]   (kw in 0..5)
which is only 1.5x the raw image bytes (the kernel was DMA-byte-bound at
~200 GB/s/core: both NCs of an SEngine share the 16 SDMA ports).
Row 96 = ones (bias via matmul), rows 97..127 = zeros: the contraction is
padded to the full 128 partitions (fast weight-path; zero rows cost no
time, LDWEIGHTS scales with columns).  Ones+zeros are written once per
slab buffer from a constant tensor; image DMAs only touch rows 0..95.

For block b (512 positions), tap kh (stationary = 128 contiguous cols at
u0 = 128b + 64kh, 16B-aligned):
  psum[m, (j,o)] += sum_k slab[k, u0 + m] * W[o, c, kh, kw-j]
Position p = 512b + 4m + j.  Channel-min is a free-dim reduce_min on DVE
batched 8 blocks (one 4-bank PSUM tile) per op, then tanh(tanh()) on ACT,
fp16 stores via SWDGE.  A short warm-up matmul burst at kernel start opens
the PE HAM clock gate (K=8/8) before the real stream begins.
"""

import sys
import types

import numpy as np

# ---------------------------------------------------------------------------
# NTFF profile hook registration (the container's antenv stub lacks
# axon_hooks; registering it enables trace=True for profiling runs).
def _install_axon_hooks():
    try:
        import antenv.axon_hooks  # noqa: F401
        return
    except ImportError:
        pass
    try:
        import antenv
        from trn_agent_boot.trn_boot import _ntff_profile_via_ctypes
    except ImportError:
        return
    mod = types.ModuleType("antenv.axon_hooks")
    _hook = [None]
    mod.set_axon_ntff_profile_hook = lambda h: _hook.__setitem__(0, h)
    mod.get_axon_ntff_profile_hook = lambda: _hook[0]
    sys.modules["antenv.axon_hooks"] = mod
    antenv.axon_hooks = mod
    try:
        mod.set_axon_ntff_profile_hook(
            _ntff_profile_via_ctypes("/opt/axon/libaxon_pjrt.so")
        )
    except Exception:
        pass


_install_axon_hooks()

import concourse.bass as bass  # noqa: E402
import concourse.tile as tile  # noqa: E402
from concourse import bacc, mybir  # noqa: E402
from concourse.bass_utils import run_bass_kernel_spmd  # noqa: E402

N_CORES = 8
IMGS_PER_CORE = 4
C_IN, H, W = 16, 256, 256
C_OUT = 64
OH = OW = 254

J = 4                  # position shifts per matmul column group
TAPS = 6               # kw taps in the contraction (0..5) = J+2
DROWS = TAPS * C_IN    # 96 data rows
KPART = 128            # contraction: 96 data + ones row 96 + zero pad
CROWS = KPART - DROWS  # 32 constant rows (ones + zeros)
NFREE = J * C_OUT      # 256 moving columns per kh tap
BLK = 128 * J          # 512 flat positions per block
IMG = H * W            # 65536
NB_IMG = 128           # blocks per image (covers all 65536 positions)
UCOLS = 128 * NB_IMG + 256  # 16640 stored columns (u = t//4), incl overhang
XPAD = 4 * UCOLS + TAPS     # padded flat image for host packing
GRP = 8                # blocks per PSUM tile / reduce op
YCOLS = NB_IMG * J     # 512 output cols per image: col = 4*b + j


def _prep_inputs(x, conv_weight, conv_bias):
    """Host-side packing: quarter-sampled fp16 slab and matmul weights.

    slab[i, kw*16+c, u] = x[i, c, 4u + kw]  (kw in 0..5, u in 0..UCOLS)
    wmov[kw*16+c, kh*256 + j*64 + o] = W[o, c, kh, kw-j] (0 outside 0..2),
    row 96 (ones) = bias at kh=0.
    """
    n = x.shape[0]
    xf = np.zeros((n, C_IN, XPAD), dtype=np.float16)
    xf[:, :, :IMG] = x.reshape(n, C_IN, IMG)
    slab = np.empty((n, DROWS, UCOLS), dtype=np.float16)
    for kw in range(TAPS):
        slab[:, kw * C_IN:(kw + 1) * C_IN, :] = (
            xf[:, :, kw:kw + 4 * UCOLS:4]
        )

    wm = np.zeros((KPART, 3, J, C_OUT), dtype=np.float32)
    for kh in range(3):
        for j in range(J):
            for kk in range(3):
                kw = j + kk
                wm[kw * C_IN:(kw + 1) * C_IN, kh, j, :] = (
                    conv_weight[:, :, kh, kk].T
                )
    wm[DROWS, 0, :, :] = conv_bias[None, :]  # bias via ones row, kh=0 only
    wmov = wm.reshape(KPART, 3 * NFREE).astype(np.float16)
    return slab, wmov


def _build_program():
    nc = bacc.Bacc(
        "TRN2", target_bir_lowering=False, debug=False, num_devices=N_CORES
    )
    f16 = mybir.dt.float16
    f32 = mybir.dt.float32

    x_d = nc.dram_tensor(
        "x", [IMGS_PER_CORE, DROWS, UCOLS], f16, kind="ExternalInput"
    )
    w_d = nc.dram_tensor("w", [KPART, 3 * NFREE], f16, kind="ExternalInput")
    # Constant slab rows 96..127: ones row (bias) then zero pad rows.
    c_d = nc.dram_tensor("c", [CROWS, UCOLS], f16, kind="ExternalInput")
    y_d = nc.dram_tensor(
        "y", [IMGS_PER_CORE, 128, YCOLS], f16, kind="ExternalOutput"
    )

    with tile.TileContext(nc) as tc:
        with (
            tc.tile_pool(name="wpool", bufs=1) as wpool,
            tc.tile_pool(name="slab", bufs=3) as slab_pool,
            tc.tile_pool(name="stage", bufs=3) as stage_pool,
            tc.tile_pool(name="outp", bufs=2) as out_pool,
            tc.tile_pool(name="psum", bufs=2, space="PSUM") as psum_pool,
        ):
            w_t = wpool.tile([KPART, 3 * NFREE], f16)
            nc.sync.dma_start(w_t[:], w_d[:])

            # Each image tile carries its own constant rows 96..127 (ones
            # row for the bias + zero pad rows): writing them into the same
            # tile gives the matmuls a real dependency edge on the fill (a
            # cross-tile "persistent" fill raced and left garbage bias).
            # Image 0 is loaded in two halves so its first blocks are ready
            # after ~half the transfer.
            def load_img(i, split=False):
                t = slab_pool.tile([KPART, UCOLS], f16, tag="slab")
                if split:
                    # quarter-loads so the first blocks are ready early;
                    # const rows go right after the first quarter
                    q = UCOLS // 4
                    nc.sync.dma_start(t[0:DROWS, 0:q], x_d[i, :, 0:q])
                    nc.sync.dma_start(t[DROWS:KPART, :], c_d[:])
                    for h in range(1, 4):
                        nc.sync.dma_start(
                            t[0:DROWS, h * q:(h + 1) * q],
                            x_d[i, :, h * q:(h + 1) * q],
                        )
                else:
                    nc.sync.dma_start(t[DROWS:KPART, :], c_d[:])
                    nc.sync.dma_start(t[0:DROWS, :], x_d[i])
                return t

            slabs = {0: load_img(0, split=True), 1: load_img(1)}

            # Warm-up matmuls on the weight tile while image 0 streams in:
            # keeps the PE HAM activity window busy so the clock gate opens
            # (K=8/8) and stays open until the real matmul stream begins.
            ps0 = psum_pool.tile([128, 4, 512], f32, tag="ps")
            for r in range(60):
                nc.tensor.matmul(
                    ps0[:, r % 4, 0:512],
                    w_t[:, 0:128],
                    w_t[:, 256:768],
                    start=True,
                    stop=True,
                )
            for i in range(IMGS_PER_CORE):
                if i + 2 < IMGS_PER_CORE:
                    slabs[i + 2] = load_img(i + 2)
                slab = slabs.pop(i)
                mn = stage_pool.tile([128, YCOLS], f32, tag="mn")
                for g in range(NB_IMG // GRP):
                    ps = psum_pool.tile([128, 4, 512], f32, tag="ps")
                    for s in range(GRP):
                        b = g * GRP + s
                        bank = s // 2
                        off = NFREE * (s % 2)
                        for kh in range(3):
                            u0 = 128 * b + 64 * kh
                            nc.tensor.matmul(
                                ps[:, bank, off:off + NFREE],
                                slab[0:KPART, u0:u0 + 128],
                                w_t[:, kh * NFREE:(kh + 1) * NFREE],
                                start=(kh == 0),
                                stop=(kh == 2),
                            )
                    # two half-group reduces: PSUM banks 0-1 free before the
                    # tail of the group, giving the PE's next-next group a
                    # wider recycle margin
                    c0 = g * GRP * J
                    for hb in range(2):
                        nc.vector.tensor_reduce(
                            mn[:, c0 + hb * 2 * GRP:
                               c0 + (hb + 1) * 2 * GRP].rearrange(
                                "p (b sj) -> p b sj", sj=2 * J
                            ),
                            ps[:, 2 * hb:2 * hb + 2, :].rearrange(
                                "p b (sj o) -> p b sj o", o=C_OUT
                            ),
                            axis=mybir.AxisListType.X,
                            op=mybir.AluOpType.min,
                        )
                t1 = stage_pool.tile([128, YCOLS], f32, tag="t1")
                th = out_pool.tile([128, YCOLS], f16, tag="th")
                nc.scalar.activation(
                    t1[:], mn[:], mybir.ActivationFunctionType.Tanh,
                )
                nc.scalar.activation(
                    th[:], t1[:], mybir.ActivationFunctionType.Tanh,
                )
                # SWDGE queue keeps output stores off the Sync FIFO so they
                # never delay the slab prefetch DMAs.
                nc.gpsimd.dma_start(y_d[i], th)
    nc.compile()
    return nc


_NC_CACHE = []


def _get_nc():
    if not _NC_CACHE:
        _NC_CACHE.append(_build_program())
    return _NC_CACHE[0]


def kernel(x, conv_weight, conv_bias, _trace=False):
    x = np.asarray(x, dtype=np.float32)
    conv_weight = np.asarray(conv_weight, dtype=np.float32)
    conv_bias = np.asarray(conv_bias, dtype=np.float32)
    n = x.shape[0]
    assert n == N_CORES * IMGS_PER_CORE

    slab, wmov = _prep_inputs(x, conv_weight, conv_bias)
    nc = _get_nc()
    cpad = np.zeros((CROWS, UCOLS), dtype=np.float16)
    cpad[0, :] = 1.0
    in_maps = [
        {
            "x": np.ascontiguousarray(
                slab[c * IMGS_PER_CORE:(c + 1) * IMGS_PER_CORE]
            ),
            "w": wmov,
            "c": cpad,
        }
        for c in range(N_CORES)
    ]
    res = run_bass_kernel_spmd(
        nc, in_maps, core_ids=list(range(N_CORES)), trace=_trace
    )
    arr = np.concatenate([r["y"] for r in res.results], axis=0)  # [32,128,512]
    # col = 4*b + j, partition = m; position p = 512*b + 4*m + j
    seg = arr.astype(np.float32).reshape(n, 128, NB_IMG, J)
    flat = seg.transpose(0, 2, 1, 3).reshape(n, IMG)
    y = flat[:, :OH * W].reshape(n, 1, OH, W)[:, :, :, :OW]
    out = np.ascontiguousarray(y)
    if _trace:
        kernel._last_result = res
    return out


# revision 31
# speedup vs baseline: 1.0408x; 1.0408x over previous
"""Trainium2 Bass kernel: conv2d(3x3,VALID) + bias -> min over C_out -> tanh(tanh).

Full-input contract: kernel(**inputs) takes the unsharded inputs
  x:           [32, 16, 256, 256] f32
  conv_weight: [64, 16, 3, 3]     f32
  conv_bias:   [64]               f32
and returns [32, 1, 254, 254] f32.

Strategy (data-parallel over batch, 4 images per core on 8 cores):
conv as matmuls with J=4 position shifts per moving column group.  The key
byte-saving: with J=4 the stationary columns are flat positions t with
t % 4 == 0 (kh offsets are 256 = 4*64, so every kh tap stays on the same
residue), so the SBUF slab only stores every 4th image column:
  slab[kw*16+c, u] = x[c, 4u + kw]   (kw in 0..5)
which is only 1.5x the raw image bytes (the kernel was DMA-byte-bound at
~200 GB/s/core: both NCs of an SEngine share the 16 SDMA ports).
Row 96 = ones (bias via matmul), rows 97..127 = zeros: the contraction is
padded to the full 128 partitions (fast weight-path; zero rows cost no
time, LDWEIGHTS scales with columns).  Ones+zeros are written once per
slab buffer from a constant tensor; image DMAs only touch rows 0..95.

For block b (512 positions), tap kh (stationary = 128 contiguous cols at
u0 = 128b + 64kh, 16B-aligned):
  psum[m, (j,o)] += sum_k slab[k, u0 + m] * W[o, c, kh, kw-j]
Position p = 512b + 4m + j.  Channel-min is a free-dim reduce_min on DVE
batched 8 blocks (one 4-bank PSUM tile) per op, then tanh(tanh()) on ACT,
fp16 stores via SWDGE.  A short warm-up matmul burst at kernel start opens
the PE HAM clock gate (K=8/8) before the real stream begins.
"""

import sys
import types

import numpy as np

# ---------------------------------------------------------------------------
# NTFF profile hook registration (the container's antenv stub lacks
# axon_hooks; registering it enables trace=True for profiling runs).
def _install_axon_hooks():
    try:
        import antenv.axon_hooks  # noqa: F401
        return
    except ImportError:
        pass
    try:
        import antenv
        from trn_agent_boot.trn_boot import _ntff_profile_via_ctypes
    except ImportError:
        return
    mod = types.ModuleType("antenv.axon_hooks")
    _hook = [None]
    mod.set_axon_ntff_profile_hook = lambda h: _hook.__setitem__(0, h)
    mod.get_axon_ntff_profile_hook = lambda: _hook[0]
    sys.modules["antenv.axon_hooks"] = mod
    antenv.axon_hooks = mod
    try:
        mod.set_axon_ntff_profile_hook(
            _ntff_profile_via_ctypes("/opt/axon/libaxon_pjrt.so")
        )
    except Exception:
        pass


_install_axon_hooks()

import concourse.bass as bass  # noqa: E402
import concourse.tile as tile  # noqa: E402
from concourse import bacc, mybir  # noqa: E402
from concourse.bass_utils import run_bass_kernel_spmd  # noqa: E402

N_CORES = 8
IMGS_PER_CORE = 4
C_IN, H, W = 16, 256, 256
C_OUT = 64
OH = OW = 254

J = 4                  # position shifts per matmul column group
TAPS = 6               # kw taps in the contraction (0..5) = J+2
DROWS = TAPS * C_IN    # 96 data rows
KPART = 128            # contraction: 96 data + ones row 96 + zero pad
CROWS = KPART - DROWS  # 32 constant rows (ones + zeros)
NFREE = J * C_OUT      # 256 moving columns per kh tap
BLK = 128 * J          # 512 flat positions per block
IMG = H * W            # 65536
NB_IMG = 127           # blocks per image: 127*512 = 65024 = exactly the
                       # valid output rows (rows 254/255 are conv garbage)
UCOLS = 128 * NB_IMG + 256  # 16512 stored columns (u = t//4), incl overhang
XPAD = 4 * UCOLS + TAPS     # padded flat image for host packing
GRP = 8                # blocks per PSUM tile / reduce op
YCOLS = NB_IMG * J     # 508 output cols per image: col = 4*b + j


def _prep_inputs(x, conv_weight, conv_bias):
    """Host-side packing: quarter-sampled fp16 slab and matmul weights.

    slab[i, kw*16+c, u] = x[i, c, 4u + kw]  (kw in 0..5, u in 0..UCOLS)
    wmov[kw*16+c, kh*256 + j*64 + o] = W[o, c, kh, kw-j] (0 outside 0..2),
    row 96 (ones) = bias at kh=0.
    """
    n = x.shape[0]
    xf = np.zeros((n, C_IN, XPAD), dtype=np.float16)
    xf[:, :, :IMG] = x.reshape(n, C_IN, IMG)
    slab = np.empty((n, DROWS, UCOLS), dtype=np.float16)
    for kw in range(TAPS):
        slab[:, kw * C_IN:(kw + 1) * C_IN, :] = (
            xf[:, :, kw:kw + 4 * UCOLS:4]
        )

    wm = np.zeros((KPART, 3, J, C_OUT), dtype=np.float32)
    for kh in range(3):
        for j in range(J):
            for kk in range(3):
                kw = j + kk
                wm[kw * C_IN:(kw + 1) * C_IN, kh, j, :] = (
                    conv_weight[:, :, kh, kk].T
                )
    wm[DROWS, 0, :, :] = conv_bias[None, :]  # bias via ones row, kh=0 only
    wmov = wm.reshape(KPART, 3 * NFREE).astype(np.float16)
    return slab, wmov


def _build_program():
    nc = bacc.Bacc(
        "TRN2", target_bir_lowering=False, debug=False, num_devices=N_CORES
    )
    f16 = mybir.dt.float16
    f32 = mybir.dt.float32

    x_d = nc.dram_tensor(
        "x", [IMGS_PER_CORE, DROWS, UCOLS], f16, kind="ExternalInput"
    )
    w_d = nc.dram_tensor("w", [KPART, 3 * NFREE], f16, kind="ExternalInput")
    # Constant slab rows 96..127: ones row (bias) then zero pad rows.
    c_d = nc.dram_tensor("c", [CROWS, UCOLS], f16, kind="ExternalInput")
    y_d = nc.dram_tensor(
        "y", [IMGS_PER_CORE, 128, YCOLS], f16, kind="ExternalOutput"
    )

    with tile.TileContext(nc) as tc:
        with (
            tc.tile_pool(name="wpool", bufs=1) as wpool,
            tc.tile_pool(name="slab", bufs=3) as slab_pool,
            tc.tile_pool(name="stage", bufs=3) as stage_pool,
            tc.tile_pool(name="outp", bufs=2) as out_pool,
            tc.tile_pool(name="psum", bufs=2, space="PSUM") as psum_pool,
        ):
            w_t = wpool.tile([KPART, 3 * NFREE], f16)
            nc.sync.dma_start(w_t[:], w_d[:])

            # Each image tile carries its own constant rows 96..127 (ones
            # row for the bias + zero pad rows): writing them into the same
            # tile gives the matmuls a real dependency edge on the fill (a
            # cross-tile "persistent" fill raced and left garbage bias).
            # Image 0 is loaded in two halves so its first blocks are ready
            # after ~half the transfer.
            def load_img(i, split=False):
                t = slab_pool.tile([KPART, UCOLS], f16, tag="slab")
                if split:
                    # quarter-loads so the first blocks are ready early;
                    # const rows go right after the first quarter
                    q = UCOLS // 4
                    nc.sync.dma_start(t[0:DROWS, 0:q], x_d[i, :, 0:q])
                    nc.sync.dma_start(t[DROWS:KPART, :], c_d[:])
                    for h in range(1, 4):
                        nc.sync.dma_start(
                            t[0:DROWS, h * q:(h + 1) * q],
                            x_d[i, :, h * q:(h + 1) * q],
                        )
                else:
                    nc.sync.dma_start(t[DROWS:KPART, :], c_d[:])
                    nc.sync.dma_start(t[0:DROWS, :], x_d[i])
                return t

            slabs = {0: load_img(0, split=True), 1: load_img(1)}

            # Warm-up matmuls on the weight tile while image 0 streams in:
            # keeps the PE HAM activity window busy so the clock gate opens
            # (K=8/8) and stays open until the real matmul stream begins.
            ps0 = psum_pool.tile([128, 4, 512], f32, tag="ps")
            for r in range(44):
                nc.tensor.matmul(
                    ps0[:, r % 4, 0:512],
                    w_t[:, 0:128],
                    w_t[:, 256:768],
                    start=True,
                    stop=True,
                )
            for i in range(IMGS_PER_CORE):
                if i + 2 < IMGS_PER_CORE:
                    slabs[i + 2] = load_img(i + 2)
                slab = slabs.pop(i)
                mn = stage_pool.tile([128, YCOLS], f32, tag="mn")
                g0 = 0
                while g0 < NB_IMG:
                    nb = min(GRP, NB_IMG - g0)
                    ps = psum_pool.tile([128, 4, 512], f32, tag="ps")
                    for s in range(nb):
                        b = g0 + s
                        bank = s // 2
                        off = NFREE * (s % 2)
                        for kh in range(3):
                            u0 = 128 * b + 64 * kh
                            nc.tensor.matmul(
                                ps[:, bank, off:off + NFREE],
                                slab[0:KPART, u0:u0 + 128],
                                w_t[:, kh * NFREE:(kh + 1) * NFREE],
                                start=(kh == 0),
                                stop=(kh == 2),
                            )
                    nfull = nb // 2
                    c0 = g0 * J
                    nc.vector.tensor_reduce(
                        mn[:, c0:c0 + 2 * J * nfull].rearrange(
                            "p (b sj) -> p b sj", sj=2 * J
                        ),
                        ps[:, 0:nfull, :].rearrange(
                            "p b (sj o) -> p b sj o", o=C_OUT
                        ),
                        axis=mybir.AxisListType.X,
                        op=mybir.AluOpType.min,
                    )
                    if nb % 2:
                        nc.vector.tensor_reduce(
                            mn[:, c0 + 2 * J * nfull:c0 + J * nb],
                            ps[:, nfull, 0:NFREE].rearrange(
                                "p (j o) -> p j o", o=C_OUT
                            ),
                            axis=mybir.AxisListType.X,
                            op=mybir.AluOpType.min,
                        )
                    g0 += nb
                t1 = stage_pool.tile([128, YCOLS], f32, tag="t1")
                th = out_pool.tile([128, YCOLS], f16, tag="th")
                nc.scalar.activation(
                    t1[:], mn[:], mybir.ActivationFunctionType.Tanh,
                )
                nc.scalar.activation(
                    th[:], t1[:], mybir.ActivationFunctionType.Tanh,
                )
                # SWDGE queue keeps output stores off the Sync FIFO so they
                # never delay the slab prefetch DMAs.
                nc.gpsimd.dma_start(y_d[i], th)
    nc.compile()
    return nc


_NC_CACHE = []


def _get_nc():
    if not _NC_CACHE:
        _NC_CACHE.append(_build_program())
    return _NC_CACHE[0]


def kernel(x, conv_weight, conv_bias, _trace=False):
    x = np.asarray(x, dtype=np.float32)
    conv_weight = np.asarray(conv_weight, dtype=np.float32)
    conv_bias = np.asarray(conv_bias, dtype=np.float32)
    n = x.shape[0]
    assert n == N_CORES * IMGS_PER_CORE

    slab, wmov = _prep_inputs(x, conv_weight, conv_bias)
    nc = _get_nc()
    cpad = np.zeros((CROWS, UCOLS), dtype=np.float16)
    cpad[0, :] = 1.0
    in_maps = [
        {
            "x": np.ascontiguousarray(
                slab[c * IMGS_PER_CORE:(c + 1) * IMGS_PER_CORE]
            ),
            "w": wmov,
            "c": cpad,
        }
        for c in range(N_CORES)
    ]
    res = run_bass_kernel_spmd(
        nc, in_maps, core_ids=list(range(N_CORES)), trace=_trace
    )
    arr = np.concatenate([r["y"] for r in res.results], axis=0)  # [32,128,512]
    # col = 4*b + j, partition = m; position p = 512*b + 4*m + j
    seg = arr.astype(np.float32).reshape(n, 128, NB_IMG, J)
    flat = seg.transpose(0, 2, 1, 3).reshape(n, NB_IMG * BLK)
    y = flat[:, :OH * W].reshape(n, 1, OH, W)[:, :, :, :OW]
    out = np.ascontiguousarray(y)
    if _trace:
        kernel._last_result = res
    return out


# revision 36
# speedup vs baseline: 1.0510x; 1.0098x over previous
"""Trainium2 Bass kernel: conv2d(3x3,VALID) + bias -> min over C_out -> tanh(tanh).

Full-input contract: kernel(**inputs) takes the unsharded inputs
  x:           [32, 16, 256, 256] f32
  conv_weight: [64, 16, 3, 3]     f32
  conv_bias:   [64]               f32
and returns [32, 1, 254, 254] f32.

Strategy (data-parallel over batch, 4 images per core on 8 cores):
conv as matmuls with J=4 position shifts per moving column group.  The key
byte-saving: with J=4 the stationary columns are flat positions t with
t % 4 == 0 (kh offsets are 256 = 4*64, so every kh tap stays on the same
residue), so the SBUF slab only stores every 4th image column:
  slab[kw*16+c, u] = x[c, 4u + kw]   (kw in 0..5)
which is only 1.5x the raw image bytes (the kernel was DMA-byte-bound at
~200 GB/s/core: both NCs of an SEngine share the 16 SDMA ports).
Row 96 = ones (bias via matmul), rows 97..127 = zeros: the contraction is
padded to the full 128 partitions (fast weight-path; zero rows cost no
time, LDWEIGHTS scales with columns).  Ones+zeros are written once per
slab buffer from a constant tensor; image DMAs only touch rows 0..95.

For block b (512 positions), tap kh (stationary = 128 contiguous cols at
u0 = 128b + 64kh, 16B-aligned):
  psum[m, (j,o)] += sum_k slab[k, u0 + m] * W[o, c, kh, kw-j]
Position p = 512b + 4m + j.  Channel-min is a free-dim reduce_min on DVE
batched 8 blocks (one 4-bank PSUM tile) per op, then tanh(tanh()) on ACT,
fp16 stores via SWDGE.  A short warm-up matmul burst at kernel start opens
the PE HAM clock gate (K=8/8) before the real stream begins.
"""

import sys
import types

import numpy as np

# ---------------------------------------------------------------------------
# NTFF profile hook registration (the container's antenv stub lacks
# axon_hooks; registering it enables trace=True for profiling runs).
def _install_axon_hooks():
    try:
        import antenv.axon_hooks  # noqa: F401
        return
    except ImportError:
        pass
    try:
        import antenv
        from trn_agent_boot.trn_boot import _ntff_profile_via_ctypes
    except ImportError:
        return
    mod = types.ModuleType("antenv.axon_hooks")
    _hook = [None]
    mod.set_axon_ntff_profile_hook = lambda h: _hook.__setitem__(0, h)
    mod.get_axon_ntff_profile_hook = lambda: _hook[0]
    sys.modules["antenv.axon_hooks"] = mod
    antenv.axon_hooks = mod
    try:
        mod.set_axon_ntff_profile_hook(
            _ntff_profile_via_ctypes("/opt/axon/libaxon_pjrt.so")
        )
    except Exception:
        pass


_install_axon_hooks()

import concourse.bass as bass  # noqa: E402
import concourse.tile as tile  # noqa: E402
from concourse import bacc, mybir  # noqa: E402
from concourse.bass_utils import run_bass_kernel_spmd  # noqa: E402

N_CORES = 8
IMGS_PER_CORE = 4
C_IN, H, W = 16, 256, 256
C_OUT = 64
OH = OW = 254

J = 4                  # position shifts per matmul column group
TAPS = 6               # kw taps in the contraction (0..5) = J+2
DROWS = TAPS * C_IN    # 96 data rows
KPART = 128            # contraction: 96 data + ones row 96 + zero pad
CROWS = KPART - DROWS  # 32 constant rows (ones + zeros)
NFREE = J * C_OUT      # 256 moving columns per kh tap
BLK = 128 * J          # 512 flat positions per block
IMG = H * W            # 65536
NB_IMG = 127           # blocks per image: 127*512 = 65024 = exactly the
                       # valid output rows (rows 254/255 are conv garbage)
UCOLS = 128 * NB_IMG + 256  # 16512 stored columns (u = t//4), incl overhang
XPAD = 4 * UCOLS + TAPS     # padded flat image for host packing
GRP = 4                # blocks per PSUM tile / reduce op (2 banks; 4 tiles
                       # in flight widen the PE->DVE PSUM recycle margin)
YCOLS = NB_IMG * J     # 508 output cols per image: col = 4*b + j


def _prep_inputs(x, conv_weight, conv_bias):
    """Host-side packing: quarter-sampled fp16 slab and matmul weights.

    slab[i, kw*16+c, u] = x[i, c, 4u + kw]  (kw in 0..5, u in 0..UCOLS)
    wmov[kw*16+c, kh*256 + j*64 + o] = W[o, c, kh, kw-j] (0 outside 0..2),
    row 96 (ones) = bias at kh=0.
    """
    n = x.shape[0]
    xf = np.zeros((n, C_IN, XPAD), dtype=np.float16)
    xf[:, :, :IMG] = x.reshape(n, C_IN, IMG)
    slab = np.empty((n, DROWS, UCOLS), dtype=np.float16)
    for kw in range(TAPS):
        slab[:, kw * C_IN:(kw + 1) * C_IN, :] = (
            xf[:, :, kw:kw + 4 * UCOLS:4]
        )

    wm = np.zeros((KPART, 3, J, C_OUT), dtype=np.float32)
    for kh in range(3):
        for j in range(J):
            for kk in range(3):
                kw = j + kk
                wm[kw * C_IN:(kw + 1) * C_IN, kh, j, :] = (
                    conv_weight[:, :, kh, kk].T
                )
    wm[DROWS, 0, :, :] = conv_bias[None, :]  # bias via ones row, kh=0 only
    wmov = wm.reshape(KPART, 3 * NFREE).astype(np.float16)
    return slab, wmov


def _build_program():
    nc = bacc.Bacc(
        "TRN2", target_bir_lowering=False, debug=False, num_devices=N_CORES
    )
    f16 = mybir.dt.float16
    f32 = mybir.dt.float32

    x_d = nc.dram_tensor(
        "x", [IMGS_PER_CORE, DROWS, UCOLS], f16, kind="ExternalInput"
    )
    w_d = nc.dram_tensor("w", [KPART, 3 * NFREE], f16, kind="ExternalInput")
    # Constant slab rows 96..127: ones row (bias) then zero pad rows.
    c_d = nc.dram_tensor("c", [CROWS, UCOLS], f16, kind="ExternalInput")
    y_d = nc.dram_tensor(
        "y", [IMGS_PER_CORE, 128, YCOLS], f16, kind="ExternalOutput"
    )

    with tile.TileContext(nc) as tc:
        with (
            tc.tile_pool(name="wpool", bufs=1) as wpool,
            tc.tile_pool(name="slab", bufs=3) as slab_pool,
            tc.tile_pool(name="stage", bufs=3) as stage_pool,
            tc.tile_pool(name="outp", bufs=2) as out_pool,
            tc.tile_pool(name="psum", bufs=4, space="PSUM") as psum_pool,
        ):
            w_t = wpool.tile([KPART, 3 * NFREE], f16)
            nc.sync.dma_start(w_t[:], w_d[:])

            # Each image tile carries its own constant rows 96..127 (ones
            # row for the bias + zero pad rows): writing them into the same
            # tile gives the matmuls a real dependency edge on the fill (a
            # cross-tile "persistent" fill raced and left garbage bias).
            # Image 0 is loaded in two halves so its first blocks are ready
            # after ~half the transfer.
            def load_img(i, split=False):
                t = slab_pool.tile([KPART, UCOLS], f16, tag="slab")
                if split:
                    # quarter-loads so the first blocks are ready early;
                    # const rows go right after the first quarter
                    q = UCOLS // 4
                    nc.sync.dma_start(t[0:DROWS, 0:q], x_d[i, :, 0:q])
                    nc.sync.dma_start(t[DROWS:KPART, :], c_d[:])
                    for h in range(1, 4):
                        nc.sync.dma_start(
                            t[0:DROWS, h * q:(h + 1) * q],
                            x_d[i, :, h * q:(h + 1) * q],
                        )
                else:
                    nc.sync.dma_start(t[DROWS:KPART, :], c_d[:])
                    nc.sync.dma_start(t[0:DROWS, :], x_d[i])
                return t

            slabs = {0: load_img(0, split=True), 1: load_img(1)}

            # Warm-up matmuls on the weight tile while image 0 streams in:
            # keeps the PE HAM activity window busy so the clock gate opens
            # (K=8/8) and stays open until the real matmul stream begins.
            ps0 = psum_pool.tile([128, 2, 512], f32, tag="ps")
            for r in range(44):
                nc.tensor.matmul(
                    ps0[:, r % 2, 0:512],
                    w_t[:, 0:128],
                    w_t[:, 256:768],
                    start=True,
                    stop=True,
                )
            for i in range(IMGS_PER_CORE):
                if i + 2 < IMGS_PER_CORE:
                    slabs[i + 2] = load_img(i + 2)
                slab = slabs.pop(i)
                mn = stage_pool.tile([128, YCOLS], f32, tag="mn")
                g0 = 0
                while g0 < NB_IMG:
                    nb = min(GRP, NB_IMG - g0)
                    ps = psum_pool.tile([128, 2, 512], f32, tag="ps")
                    for s in range(nb):
                        b = g0 + s
                        bank = s // 2
                        off = NFREE * (s % 2)
                        for kh in range(3):
                            u0 = 128 * b + 64 * kh
                            nc.tensor.matmul(
                                ps[:, bank, off:off + NFREE],
                                slab[0:KPART, u0:u0 + 128],
                                w_t[:, kh * NFREE:(kh + 1) * NFREE],
                                start=(kh == 0),
                                stop=(kh == 2),
                            )
                    nfull = nb // 2
                    c0 = g0 * J
                    nc.vector.tensor_reduce(
                        mn[:, c0:c0 + 2 * J * nfull].rearrange(
                            "p (b sj) -> p b sj", sj=2 * J
                        ),
                        ps[:, 0:nfull, :].rearrange(
                            "p b (sj o) -> p b sj o", o=C_OUT
                        ),
                        axis=mybir.AxisListType.X,
                        op=mybir.AluOpType.min,
                    )
                    if nb % 2:
                        nc.vector.tensor_reduce(
                            mn[:, c0 + 2 * J * nfull:c0 + J * nb],
                            ps[:, nfull, 0:NFREE].rearrange(
                                "p (j o) -> p j o", o=C_OUT
                            ),
                            axis=mybir.AxisListType.X,
                            op=mybir.AluOpType.min,
                        )
                    g0 += nb
                t1 = stage_pool.tile([128, YCOLS], f32, tag="t1")
                th = out_pool.tile([128, YCOLS], f16, tag="th")
                # two halves so tanh/store of the first half overlap the
                # last groups' matmuls/reduces (shrinks the drain tail)
                hc = (YCOLS // 2) & ~3
                for lo, hi in ((0, hc), (hc, YCOLS)):
                    nc.scalar.activation(
                        t1[:, lo:hi], mn[:, lo:hi],
                        mybir.ActivationFunctionType.Tanh,
                    )
                    nc.scalar.activation(
                        th[:, lo:hi], t1[:, lo:hi],
                        mybir.ActivationFunctionType.Tanh,
                    )
                    # SWDGE queue keeps output stores off the Sync FIFO so
                    # they never delay the slab prefetch DMAs.
                    nc.gpsimd.dma_start(y_d[i, :, lo:hi], th[:, lo:hi])
    nc.compile()
    return nc


_NC_CACHE = []


def _get_nc():
    if not _NC_CACHE:
        _NC_CACHE.append(_build_program())
    return _NC_CACHE[0]


def kernel(x, conv_weight, conv_bias, _trace=False):
    x = np.asarray(x, dtype=np.float32)
    conv_weight = np.asarray(conv_weight, dtype=np.float32)
    conv_bias = np.asarray(conv_bias, dtype=np.float32)
    n = x.shape[0]
    assert n == N_CORES * IMGS_PER_CORE

    slab, wmov = _prep_inputs(x, conv_weight, conv_bias)
    nc = _get_nc()
    cpad = np.zeros((CROWS, UCOLS), dtype=np.float16)
    cpad[0, :] = 1.0
    in_maps = [
        {
            "x": np.ascontiguousarray(
                slab[c * IMGS_PER_CORE:(c + 1) * IMGS_PER_CORE]
            ),
            "w": wmov,
            "c": cpad,
        }
        for c in range(N_CORES)
    ]
    res = run_bass_kernel_spmd(
        nc, in_maps, core_ids=list(range(N_CORES)), trace=_trace
    )
    arr = np.concatenate([r["y"] for r in res.results], axis=0)  # [32,128,512]
    # col = 4*b + j, partition = m; position p = 512*b + 4*m + j
    seg = arr.astype(np.float32).reshape(n, 128, NB_IMG, J)
    flat = seg.transpose(0, 2, 1, 3).reshape(n, NB_IMG * BLK)
    y = flat[:, :OH * W].reshape(n, 1, OH, W)[:, :, :, :OW]
    out = np.ascontiguousarray(y)
    if _trace:
        kernel._last_result = res
    return out


# revision 39
# speedup vs baseline: 1.0860x; 1.0333x over previous
"""Trainium2 Bass kernel: conv2d(3x3,VALID) + bias -> min over C_out -> tanh(tanh).

Full-input contract: kernel(**inputs) takes the unsharded inputs
  x:           [32, 16, 256, 256] f32
  conv_weight: [64, 16, 3, 3]     f32
  conv_bias:   [64]               f32
and returns [32, 1, 254, 254] f32.

Strategy (data-parallel over batch, 4 images per core on 8 cores):
conv as matmuls with J=4 position shifts per moving column group.  The key
byte-saving: with J=4 the stationary columns are flat positions t with
t % 4 == 0 (kh offsets are 256 = 4*64, so every kh tap stays on the same
residue), so the SBUF slab only stores every 4th image column:
  slab[kw*16+c, u] = x[c, 4u + kw]   (kw in 0..5)
which is only 1.5x the raw image bytes (the kernel was DMA-byte-bound at
~200 GB/s/core: both NCs of an SEngine share the 16 SDMA ports).
Row 96 = ones (bias via matmul), rows 97..127 = zeros: the contraction is
padded to the full 128 partitions (fast weight-path; zero rows cost no
time, LDWEIGHTS scales with columns).  Ones+zeros are written once per
slab buffer from a constant tensor; image DMAs only touch rows 0..95.

For block b (512 positions), tap kh (stationary = 128 contiguous cols at
u0 = 128b + 64kh, 16B-aligned):
  psum[m, (j,o)] += sum_k slab[k, u0 + m] * W[o, c, kh, kw-j]
Position p = 512b + 4m + j.  Channel-min is a free-dim reduce_min on DVE
batched 8 blocks (one 4-bank PSUM tile) per op, then tanh(tanh()) on ACT,
fp16 stores via SWDGE.  A short warm-up matmul burst at kernel start opens
the PE HAM clock gate (K=8/8) before the real stream begins.
"""

import sys
import types

import numpy as np

# ---------------------------------------------------------------------------
# NTFF profile hook registration (the container's antenv stub lacks
# axon_hooks; registering it enables trace=True for profiling runs).
def _install_axon_hooks():
    try:
        import antenv.axon_hooks  # noqa: F401
        return
    except ImportError:
        pass
    try:
        import antenv
        from trn_agent_boot.trn_boot import _ntff_profile_via_ctypes
    except ImportError:
        return
    mod = types.ModuleType("antenv.axon_hooks")
    _hook = [None]
    mod.set_axon_ntff_profile_hook = lambda h: _hook.__setitem__(0, h)
    mod.get_axon_ntff_profile_hook = lambda: _hook[0]
    sys.modules["antenv.axon_hooks"] = mod
    antenv.axon_hooks = mod
    try:
        mod.set_axon_ntff_profile_hook(
            _ntff_profile_via_ctypes("/opt/axon/libaxon_pjrt.so")
        )
    except Exception:
        pass


_install_axon_hooks()

import concourse.bass as bass  # noqa: E402
import concourse.tile as tile  # noqa: E402
from concourse import bacc, mybir  # noqa: E402
from concourse.bass_utils import run_bass_kernel_spmd  # noqa: E402

N_CORES = 8
IMGS_PER_CORE = 4
C_IN, H, W = 16, 256, 256
C_OUT = 64
OH = OW = 254

J = 4                  # position shifts per matmul column group
TAPS = 6               # kw taps in the contraction (0..5) = J+2
DROWS = TAPS * C_IN    # 96 data rows
KPART = 128            # contraction: 96 data + ones row 96 + zero pad
CROWS = KPART - DROWS  # 32 constant rows (ones + zeros)
NFREE = J * C_OUT      # 256 moving columns per kh tap
BLK = 128 * J          # 512 flat positions per block
IMG = H * W            # 65536
NB_IMG = 127           # blocks per image: 127*512 = 65024 = exactly the
                       # valid output rows (rows 254/255 are conv garbage)
UCOLS = 128 * NB_IMG + 256  # 16512 stored columns (u = t//4), incl overhang
XPAD = 4 * UCOLS + TAPS     # padded flat image for host packing
GRP = 4                # blocks per PSUM tile / reduce op (2 banks; 4 tiles
                       # in flight widen the PE->DVE PSUM recycle margin)
YCOLS = NB_IMG * J     # 508 output cols per image: col = 4*b + j


def _prep_inputs(x, conv_weight, conv_bias):
    """Host-side packing: quarter-sampled fp16 slab and matmul weights.

    slab[i, kw*16+c, u] = x[i, c, 4u + kw]  (kw in 0..5, u in 0..UCOLS)
    wmov[kw*16+c, kh*256 + j*64 + o] = W[o, c, kh, kw-j] (0 outside 0..2),
    row 96 (ones) = bias at kh=0.
    """
    n = x.shape[0]
    xf = np.zeros((n, C_IN, XPAD), dtype=np.float16)
    xf[:, :, :IMG] = x.reshape(n, C_IN, IMG)
    slab = np.empty((n, DROWS, UCOLS), dtype=np.float16)
    for kw in range(TAPS):
        slab[:, kw * C_IN:(kw + 1) * C_IN, :] = (
            xf[:, :, kw:kw + 4 * UCOLS:4]
        )

    wm = np.zeros((KPART, 3, J, C_OUT), dtype=np.float32)
    for kh in range(3):
        for j in range(J):
            for kk in range(3):
                kw = j + kk
                wm[kw * C_IN:(kw + 1) * C_IN, kh, j, :] = (
                    conv_weight[:, :, kh, kk].T
                )
    wm[DROWS, 0, :, :] = conv_bias[None, :]  # bias via ones row, kh=0 only
    wmov = wm.reshape(KPART, 3 * NFREE).astype(np.float16)
    return slab, wmov


def _build_program():
    nc = bacc.Bacc(
        "TRN2", target_bir_lowering=False, debug=False, num_devices=N_CORES
    )
    f16 = mybir.dt.float16
    f32 = mybir.dt.float32

    x_d = nc.dram_tensor(
        "x", [IMGS_PER_CORE, DROWS, UCOLS], f16, kind="ExternalInput"
    )
    w_d = nc.dram_tensor("w", [KPART, 3 * NFREE], f16, kind="ExternalInput")
    # Constant slab rows 96..127: ones row (bias) then zero pad rows.
    c_d = nc.dram_tensor("c", [CROWS, UCOLS], f16, kind="ExternalInput")
    y_d = nc.dram_tensor(
        "y", [IMGS_PER_CORE, 128, YCOLS], f16, kind="ExternalOutput"
    )

    with tile.TileContext(nc) as tc:
        with (
            tc.tile_pool(name="wpool", bufs=1) as wpool,
            tc.tile_pool(name="slab", bufs=3) as slab_pool,
            tc.tile_pool(name="stage", bufs=3) as stage_pool,
            tc.tile_pool(name="outp", bufs=2) as out_pool,
            tc.tile_pool(name="psum", bufs=4, space="PSUM") as psum_pool,
        ):
            w_t = wpool.tile([KPART, 3 * NFREE], f16)
            nc.sync.dma_start(w_t[:], w_d[:])

            # Each image tile carries its own constant rows 96..127 (ones
            # row for the bias + zero pad rows): writing them into the same
            # tile gives the matmuls a real dependency edge on the fill (a
            # cross-tile "persistent" fill raced and left garbage bias).
            # Image 0 is loaded in two halves so its first blocks are ready
            # after ~half the transfer.
            def load_img(i, split=False):
                t = slab_pool.tile([KPART, UCOLS], f16, tag="slab")
                if split:
                    # lead with a tiny chunk (first 10 blocks' worth of
                    # slab + const rows) so the first matmuls can start
                    # ~3us in, then stream the rest in three pieces
                    q0 = 2560
                    nc.sync.dma_start(
                        t[DROWS:KPART, 0:q0], c_d[:, 0:q0]
                    )
                    nc.sync.dma_start(t[0:DROWS, 0:q0], x_d[i, :, 0:q0])
                    nc.sync.dma_start(
                        t[DROWS:KPART, q0:UCOLS], c_d[:, q0:UCOLS]
                    )
                    q = (UCOLS - q0) // 3 + 1
                    for h in range(3):
                        lo = q0 + h * q
                        hi = min(UCOLS, lo + q)
                        nc.sync.dma_start(
                            t[0:DROWS, lo:hi], x_d[i, :, lo:hi]
                        )
                else:
                    nc.sync.dma_start(t[DROWS:KPART, :], c_d[:])
                    nc.sync.dma_start(t[0:DROWS, :], x_d[i])
                return t

            slabs = {0: load_img(0, split=True), 1: load_img(1)}

            # Warm-up matmuls on the weight tile while image 0 streams in:
            # keeps the PE HAM activity window busy so the clock gate opens
            # (K=8/8) and stays open until the real matmul stream begins.
            ps0 = psum_pool.tile([128, 2, 512], f32, tag="ps")
            for r in range(12):
                nc.tensor.matmul(
                    ps0[:, r % 2, 0:512],
                    w_t[:, 0:128],
                    w_t[:, 256:768],
                    start=True,
                    stop=True,
                )
            for i in range(IMGS_PER_CORE):
                if i + 2 < IMGS_PER_CORE:
                    slabs[i + 2] = load_img(i + 2)
                slab = slabs.pop(i)
                mn = stage_pool.tile([128, YCOLS], f32, tag="mn")
                g0 = 0
                while g0 < NB_IMG:
                    nb = min(GRP, NB_IMG - g0)
                    ps = psum_pool.tile([128, 2, 512], f32, tag="ps")
                    for s in range(nb):
                        b = g0 + s
                        bank = s // 2
                        off = NFREE * (s % 2)
                        for kh in range(3):
                            u0 = 128 * b + 64 * kh
                            nc.tensor.matmul(
                                ps[:, bank, off:off + NFREE],
                                slab[0:KPART, u0:u0 + 128],
                                w_t[:, kh * NFREE:(kh + 1) * NFREE],
                                start=(kh == 0),
                                stop=(kh == 2),
                            )
                    nfull = nb // 2
                    c0 = g0 * J
                    nc.vector.tensor_reduce(
                        mn[:, c0:c0 + 2 * J * nfull].rearrange(
                            "p (b sj) -> p b sj", sj=2 * J
                        ),
                        ps[:, 0:nfull, :].rearrange(
                            "p b (sj o) -> p b sj o", o=C_OUT
                        ),
                        axis=mybir.AxisListType.X,
                        op=mybir.AluOpType.min,
                    )
                    if nb % 2:
                        nc.vector.tensor_reduce(
                            mn[:, c0 + 2 * J * nfull:c0 + J * nb],
                            ps[:, nfull, 0:NFREE].rearrange(
                                "p (j o) -> p j o", o=C_OUT
                            ),
                            axis=mybir.AxisListType.X,
                            op=mybir.AluOpType.min,
                        )
                    g0 += nb
                t1 = stage_pool.tile([128, YCOLS], f32, tag="t1")
                th = out_pool.tile([128, YCOLS], f16, tag="th")
                # two halves so tanh/store of the first half overlap the
                # last groups' matmuls/reduces (shrinks the drain tail)
                hc = (YCOLS // 2) & ~3
                for lo, hi in ((0, hc), (hc, YCOLS)):
                    nc.scalar.activation(
                        t1[:, lo:hi], mn[:, lo:hi],
                        mybir.ActivationFunctionType.Tanh,
                    )
                    nc.scalar.activation(
                        th[:, lo:hi], t1[:, lo:hi],
                        mybir.ActivationFunctionType.Tanh,
                    )
                    # SWDGE queue keeps output stores off the Sync FIFO so
                    # they never delay the slab prefetch DMAs.
                    nc.gpsimd.dma_start(y_d[i, :, lo:hi], th[:, lo:hi])
    nc.compile()
    return nc


_NC_CACHE = []


def _get_nc():
    if not _NC_CACHE:
        _NC_CACHE.append(_build_program())
    return _NC_CACHE[0]


def kernel(x, conv_weight, conv_bias, _trace=False):
    x = np.asarray(x, dtype=np.float32)
    conv_weight = np.asarray(conv_weight, dtype=np.float32)
    conv_bias = np.asarray(conv_bias, dtype=np.float32)
    n = x.shape[0]
    assert n == N_CORES * IMGS_PER_CORE

    slab, wmov = _prep_inputs(x, conv_weight, conv_bias)
    nc = _get_nc()
    cpad = np.zeros((CROWS, UCOLS), dtype=np.float16)
    cpad[0, :] = 1.0
    in_maps = [
        {
            "x": np.ascontiguousarray(
                slab[c * IMGS_PER_CORE:(c + 1) * IMGS_PER_CORE]
            ),
            "w": wmov,
            "c": cpad,
        }
        for c in range(N_CORES)
    ]
    res = run_bass_kernel_spmd(
        nc, in_maps, core_ids=list(range(N_CORES)), trace=_trace
    )
    arr = np.concatenate([r["y"] for r in res.results], axis=0)  # [32,128,512]
    # col = 4*b + j, partition = m; position p = 512*b + 4*m + j
    seg = arr.astype(np.float32).reshape(n, 128, NB_IMG, J)
    flat = seg.transpose(0, 2, 1, 3).reshape(n, NB_IMG * BLK)
    y = flat[:, :OH * W].reshape(n, 1, OH, W)[:, :, :, :OW]
    out = np.ascontiguousarray(y)
    if _trace:
        kernel._last_result = res
    return out
